# revision 1
# baseline (speedup 1.0000x reference)
"""AtnConv (contextual attention) Trainium2 Bass kernel, 8-core SPMD.

Decomposition (per batch b, L=2304=48*48 patches, C=128):
  P  = im2col3x3(x2_pad)                    [1152, L]
  logits[p, l] = (P[:,p]*10*ma[p]) . (P[:,l]*mm[l]/max(|P[:,l]|,1e-4))
  sm = softmax over l (free dim)            [p, l]
  Yt = max(sm * post[l,p], 1e-8),  post = (1+0.5*mask_c)*mm[l]*ma[p]
  col[p, :] = Yt @ RW,  RW = im2col4x4s2(x1_pad)  [L, 2048]
  y = col2im(col)/4 ; out = concat_g relu(dilated_conv3x3(y, fuse_w[g]) + fuse_b[g])

Sharding: 8 cores = 2 batches x 4 chunks of 576 p-columns (padded to 640).
Kernel 1 (per core): Gram matmul -> softmax -> post-mul -> PE transpose ->
second matmul -> col chunk. Kernel 2 (per core = batch x row-quarter):
4 dilated fuse convs on a 40-row halo slab. Host does im2col / col2im /
scaling prep (pure indexing + tiny elementwise only).
"""
import numpy as np
import ml_dtypes
from contextlib import ExitStack

import concourse.bass as bass
import concourse.bacc as bacc
import concourse.tile as tile
import concourse.mybir as mybir
from concourse import bass_utils
from concourse.bass import ts
from concourse.masks import make_identity

BF16 = mybir.dt.bfloat16
F32 = mybir.dt.float32
H = W = 48
L = H * W           # 2304
C = 128
CHUNK = 576         # L/4 p-columns per core
CHUNKP = 640        # padded to 5*128
SCALE = 10.0
DILS = (1, 2, 4, 8)
NT = [512, 512, 512, 512, 256]   # l-dim tiling of 2304

_cache = {}


# ---------------------------------------------------------------- host prep
def _im2col3(x):
    # x [C,H,W] -> [C*9, H*W] with zero pad 1 (c-major, then ki, kj)
    Cc, Hh, Ww = x.shape
    xp = np.pad(x, ((0, 0), (1, 1), (1, 1)))
    cols = np.empty((Cc, 3, 3, Hh, Ww), np.float32)
    for ki in range(3):
        for kj in range(3):
            cols[:, ki, kj] = xp[:, ki:ki + Hh, kj:kj + Ww]
    return cols.reshape(Cc * 9, Hh * Ww)


def _im2col4s2(x):
    # x [C,96,96] -> [L, C*16], k=4 stride 2 pad 1
    Cc = x.shape[0]
    xp = np.pad(x, ((0, 0), (1, 1), (1, 1)))
    out = np.empty((H, W, Cc, 4, 4), np.float32)
    for ki in range(4):
        for kj in range(4):
            out[:, :, :, ki, kj] = xp[:, ki:ki + 2 * H:2, kj:kj + 2 * W:2].transpose(1, 2, 0)
    return out.reshape(L, Cc * 16)


def _neighbor_mask():
    M = np.zeros((L, L), np.float32)
    p = np.arange(L)
    pi, pj = p // W, p % W
    for off, sel in ((-1, pj >= 1), (1, pj <= W - 2), (W, pi <= H - 2), (-W, pi >= 1)):
        M[p[sel] + off, p[sel]] = 1.0
    return M


def _col2im(col):
    # col [L, C*16] -> [C, 96, 96] scatter-add (stride 2, pad 1)
    colr = col.reshape(H, W, C, 4, 4)
    out = np.zeros((C, 99, 99), np.float32)
    for ki in range(4):
        for kj in range(4):
            out[:, ki:ki + 96:2, kj:kj + 96:2] += colr[:, :, :, ki, kj].transpose(2, 0, 1)
    return out[:, 1:97, 1:97]


def _pack_part(a, p):
    # [N, F] -> [p, N//p, F] partition-major packing (row r = t*p + pp)
    n, f = a.shape
    return np.ascontiguousarray(a.reshape(n // p, p, f).transpose(1, 0, 2))


# ---------------------------------------------------------------- kernels
def _build_main():
    nc = bacc.Bacc("TRN2", target_bir_lowering=False, debug=False, num_devices=8)
    lhs = nc.dram_tensor("lhsP", [128, 9, CHUNKP], BF16, kind="ExternalInput").ap()
    rhs = nc.dram_tensor("rhsP", [128, 9, L], BF16, kind="ExternalInput").ap()
    post = nc.dram_tensor("post", [128, 5, L], BF16, kind="ExternalInput").ap()
    rw = nc.dram_tensor("rw", [128, 18, 2048], BF16, kind="ExternalInput").ap()
    col = nc.dram_tensor("col", [128, 5, 2048], F32, kind="ExternalOutput").ap()

    with tile.TileContext(nc) as tc, ExitStack() as ctx:
        const = ctx.enter_context(tc.tile_pool(name="const", bufs=1))
        ident = const.tile([128, 128], BF16)
        make_identity(nc, ident)
        ins = ctx.enter_context(tc.tile_pool(name="ins", bufs=1))
        s_lhs = ins.tile([128, 9, CHUNKP], BF16, tag="lhs")
        s_rhs = ins.tile([128, 9, L], BF16, tag="rhs")
        s_post = ins.tile([128, 5, L], BF16, tag="post")
        s_rw = ins.tile([128, 18, 2048], BF16, tag="rw")
        # split DMAs so the first matmuls' deps land early (lhs m=0 first)
        for m in range(5):
            nc.sync.dma_start(s_lhs[:, :, ts(m, 128)], lhs[:, :, ts(m, 128)])
        for k in range(9):  # first l-slice per k-tile: first matmul starts early
            nc.sync.dma_start(s_rhs[:, k, 0:512], rhs[:, k, 0:512])
        off = 512
        for sz in NT[1:]:
            nc.sync.dma_start(s_rhs[:, :, off:off + sz], rhs[:, :, off:off + sz])
            off += sz
        for m in range(5):
            nc.sync.dma_start(s_post[:, m, :], post[:, m, :])
        for nn in range(4):
            nc.sync.dma_start(s_rw[:, :, ts(nn, 512)], rw[:, :, ts(nn, 512)])

        # PSUM banks: Ysl 5 + P2 1 + pT 2 = 8
        psum1 = ctx.enter_context(tc.tile_pool(name="psum1", bufs=5, space="PSUM"))
        psum2 = ctx.enter_context(tc.tile_pool(name="psum2", bufs=2, space="PSUM"))
        psumT = ctx.enter_context(tc.tile_pool(name="psumT", bufs=1, space="PSUM"))
        work = ctx.enter_context(tc.tile_pool(name="work", bufs=2))
        stats = ctx.enter_context(tc.tile_pool(name="stats", bufs=3))

        def mm1_stage(m):
            ysl = []
            mx = stats.tile([128, 5], F32, tag="mx")
            for n, sz in enumerate(NT):
                off = n * 512
                Y = psum1.tile([128, 512], F32, tag="Y")
                for k in range(9):
                    nc.tensor.matmul(
                        Y[:, :sz],
                        lhsT=s_lhs[:, k, ts(m, 128)],
                        rhs=s_rhs[:, k, off:off + sz],
                        start=(k == 0), stop=(k == 8),
                    )
                # eager per-slice max (DVE runs under following matmuls)
                nc.vector.reduce_max(mx[:, n:n + 1], Y[:, :sz],
                                     axis=mybir.AxisListType.X)
                ysl.append(Y)
            return ysl, mx

        def rest_stage(m, ysl, mx):
            negm = stats.tile([128, 1], F32, tag="negm")
            nc.vector.tensor_reduce(negm, mx, axis=mybir.AxisListType.X,
                                    op=mybir.AluOpType.max, negate=True)
            sums = stats.tile([128, 5], F32, tag="sums")
            yt1 = work.tile([128, L], BF16, tag="yt1")
            for n, sz in enumerate(NT):
                off = n * 512
                ye = work.tile([128, 512], BF16, tag="ye")
                nc.scalar.activation(ye[:, :sz], ysl[n][:, :sz],
                                     mybir.ActivationFunctionType.Exp,
                                     bias=negm, accum_out=sums[:, n:n + 1])
                nc.vector.tensor_mul(yt1[:, off:off + sz], ye[:, :sz],
                                     s_post[:, m, off:off + sz])
            stot = stats.tile([128, 1], F32, tag="stot")
            nc.vector.reduce_sum(stot, sums, axis=mybir.AxisListType.X)
            rcp = stats.tile([128, 1], F32, tag="rcp")
            nc.vector.reciprocal(rcp, stot)
            yt = work.tile([128, L], BF16, tag="yt")
            nc.vector.tensor_scalar(yt, yt1, scalar1=rcp, scalar2=1e-8,
                                    op0=mybir.AluOpType.mult,
                                    op1=mybir.AluOpType.max)
            # transpose 18 [128,128] blocks: yt [p, l] -> ytT [l, p]
            # batch 4 transposes per PSUM tile -> one evict copy each
            ytT = work.tile([128, 18, 128], BF16, tag="ytT")
            for t0 in range(0, 18, 4):
                nb = min(4, 18 - t0)
                pT = psumT.tile([128, 4, 128], BF16, tag="pT")
                for k in range(t0, t0 + nb):
                    nc.tensor.transpose(pT[:, k - t0, :], yt[:, ts(k, 128)], ident)
                nc.any.tensor_copy(ytT[:, t0:t0 + nb, :], pT[:, :nb, :])
            # second matmul: col[p, co] = sum_l Yt[l, p] * RW[l, co]
            colm = work.tile([128, 2048], F32, tag="colm")
            for nn in range(4):
                P2 = psum2.tile([128, 512], F32, tag="P2")
                for k in range(18):
                    nc.tensor.matmul(P2, lhsT=ytT[:, k, :],
                                     rhs=s_rw[:, k, ts(nn, 512)],
                                     start=(k == 0), stop=(k == 17))
                nc.any.tensor_copy(colm[:, ts(nn, 512)], P2)
                nc.sync.dma_start(col[:, m, ts(nn, 512)], colm[:, ts(nn, 512)])

        # software pipeline: mm1(m+1) overlaps softmax/transpose/mm2 of m
        prev = None
        for m in range(5):
            cur = mm1_stage(m)
            if prev is not None:
                rest_stage(m - 1, *prev)
            prev = cur
        rest_stage(4, *prev)
    nc.compile()
    return nc


def _build_fuse():
    nc = bacc.Bacc("TRN2", target_bir_lowering=False, debug=False, num_devices=8)
    y = nc.dram_tensor("yslab", [128, 40, 112], BF16, kind="ExternalInput").ap()
    fw = nc.dram_tensor("fw", [128, 4, 9, 16], BF16, kind="ExternalInput").ap()
    fb = nc.dram_tensor("fb", [16, 4], F32, kind="ExternalInput").ap()
    fo = nc.dram_tensor("fo", [16, 4, 24 * 96], F32, kind="ExternalOutput").ap()

    RT = [(0, 5), (5, 5), (10, 5), (15, 5), (20, 4)]
    with tile.TileContext(nc) as tc, ExitStack() as ctx:
        ins = ctx.enter_context(tc.tile_pool(name="ins", bufs=1))
        s_w = ins.tile([128, 4, 9, 16], BF16, tag="w")
        nc.sync.dma_start(s_w, fw)
        s_b = ins.tile([16, 4], F32, tag="b")
        nc.sync.dma_start(s_b, fb)
        s_y = ins.tile([128, 40, 112], BF16, tag="y")
        for rc in range(5):
            nc.sync.dma_start(s_y[:, 8 * rc:8 * rc + 8, :], y[:, 8 * rc:8 * rc + 8, :])
        psum = ctx.enter_context(tc.tile_pool(name="psum", bufs=8, space="PSUM"))
        work = ctx.enter_context(tc.tile_pool(name="work", bufs=8))
        for g in range(4):
            d = DILS[g]
            for r0, nr in RT:
                ps = psum.tile([16, 512], F32, tag="ps")
                n = nr * 96
                first = True
                for ki in range(3):
                    for kj in range(3):
                        u0 = 8 + r0 + d * (ki - 1)
                        v0 = 8 + d * (kj - 1)
                        nc.tensor.matmul(
                            ps[:, :n],
                            lhsT=s_w[:, g, ki * 3 + kj, :],
                            rhs=s_y[:, u0:u0 + nr, v0:v0 + 96],
                            start=first, stop=(ki == 2 and kj == 2),
                        )
                        first = False
                ob = work.tile([16, 512], F32, tag="ob")
                nc.scalar.activation(ob[:, :n], ps[:, :n],
                                     mybir.ActivationFunctionType.Relu,
                                     bias=s_b[:, g:g + 1])
                nc.sync.dma_start(fo[:, g, r0 * 96:r0 * 96 + n], ob[:, :n])
    nc.compile()
    return nc


def _get(name, builder):
    if name not in _cache:
        _cache[name] = builder()
    return _cache[name]


# ---------------------------------------------------------------- entry
def kernel(x1, x2, mask, mask_all, fuse_w, fuse_b, _collect=None):
    x1 = np.asarray(x1, np.float32)
    x2 = np.asarray(x2, np.float32)
    mask = np.asarray(mask, np.float32)
    mask_all = np.asarray(mask_all, np.float32)
    fuse_w = np.asarray(fuse_w, np.float32)
    fuse_b = np.asarray(fuse_b, np.float32)
    N = x1.shape[0]
    bf = ml_dtypes.bfloat16

    NB = _neighbor_mask()  # [L, L]
    in_maps = []
    for b in range(N):
        P = _im2col3(x2[b])                       # [1152, L]
        norms = np.sqrt((P * P).sum(0))
        mp = _im2col3(mask[b])                    # [9, L]
        mm = (mp.mean(0) == 0.0).astype(np.float32)
        ma = mask_all[b, 0].reshape(L)
        lhs_full = P * (SCALE * ma)[None, :]      # scale col p
        rhs_full = P * (mm / np.maximum(norms, 1e-4))[None, :]
        rhs_r = _pack_part(rhs_full, 128).astype(bf)      # [128, 9, L]
        RW = _im2col4s2(x1[b])                    # [L, 2048]
        rw_r = _pack_part(RW, 128).astype(bf)             # [128, 18, 2048]
        postF = (1.0 + 0.5 * NB) * mm[:, None] * ma[None, :]  # [l, p]
        for j in range(4):
            sl = slice(j * CHUNK, (j + 1) * CHUNK)
            lhs_c = np.zeros((1152, CHUNKP), np.float32)
            lhs_c[:, :CHUNK] = lhs_full[:, sl]
            post_c = np.zeros((CHUNKP, L), np.float32)
            post_c[:CHUNK] = postF.T[sl]          # [p, l]
            in_maps.append({
                "lhsP": _pack_part(lhs_c, 128).astype(bf),
                "rhsP": rhs_r,
                "post": _pack_part(post_c, 128).astype(bf),
                "rw": rw_r,
            })
    nc1 = _get("main", _build_main)
    res1 = bass_utils.run_bass_kernel_spmd(nc1, in_maps, core_ids=list(range(8)))
    if _collect is not None:
        _collect.append(res1)

    ys = []
    for b in range(N):
        cols = []
        for j in range(4):
            r = res1.results[b * 4 + j]["col"]     # [128, 5, 2048]
            cols.append(r.transpose(1, 0, 2).reshape(CHUNKP, 2048)[:CHUNK])
        col = np.concatenate(cols, 0)              # [L, 2048]
        ys.append(_col2im(col) / 4.0)
    y = np.stack(ys)                               # [N, 128, 96, 96]

    fw_r = np.ascontiguousarray(
        fuse_w.transpose(2, 0, 3, 4, 1).reshape(128, 4, 9, 16)).astype(bf)
    fb_r = np.ascontiguousarray(fuse_b.T).astype(np.float32)  # [16, 4]
    in_maps2 = []
    for b in range(N):
        yp = np.pad(y[b], ((0, 0), (8, 8), (8, 8))).astype(bf)  # [128,112,112]
        for q in range(4):
            in_maps2.append({
                "yslab": np.ascontiguousarray(yp[:, 24 * q:24 * q + 40, :]),
                "fw": fw_r, "fb": fb_r,
            })
    nc2 = _get("fuse", _build_fuse)
    res2 = bass_utils.run_bass_kernel_spmd(nc2, in_maps2, core_ids=list(range(8)))
    if _collect is not None:
        _collect.append(res2)

    out = np.empty((N, 64, 96, 96), np.float32)
    for b in range(N):
        for q in range(4):
            r = res2.results[b * 4 + q]            # [16, 4, 2304]
            o = r["fo"].reshape(16, 4, 24, 96)
            out[b, :, 24 * q:24 * q + 24, :] = o.transpose(1, 0, 2, 3).reshape(64, 24, 96)
    return out



# revision 29
# speedup vs baseline: 1.2801x; 1.2801x over previous
"""AtnConv (contextual attention) Trainium2 Bass kernel, 8-core SPMD.

Decomposition (per batch b, L=2304=48*48 patches, C=128):
  P  = im2col3x3(x2_pad)                    [1152, L]
  logits[p, l] = (P[:,p]*10*ma[p]) . (P[:,l]*mm[l]/max(|P[:,l]|,1e-4))
  sm = softmax over l (free dim)            [p, l]
  Yt = max(sm * post[l,p], 1e-8),  post = (1+0.5*mask_c)*mm[l]*ma[p]
  col[p, :] = Yt @ RW,  RW = im2col4x4s2(x1_pad)  [L, 2048]
  y = col2im(col)/4 ; out = concat_g relu(dilated_conv3x3(y, fuse_w[g]) + fuse_b[g])

Key optimizations over the naive mapping:
  * l-compaction: columns l with mm[l]=0 (hole patches) have logits == 0
    exactly (the mask is folded into the rhs scaling), so they are dropped
    from the Gram matmul and from the second matmul's contraction.  Their
    softmax-denominator contribution is exp(-max) each, added analytically
    on-device; their 1e-8*RW[l] contribution to col is a p-independent
    vector added on host.
  * fp8 (e4m3, DoubleRow) Gram matmul for p-rows whose self-match column is
    valid: the softmax there is dominated by the exact self-match logit
    (~34x larger than cross terms), so fp8 logit noise is invisible in the
    output.  Hole p-rows (soft attention) stay bf16.  Host permutes p so
    fp8/bf16 rows land in separate 128-col stages.
  * mm2 runs on 4 PSUM banks in a single k-loop; the next-next stage's mm1
    is interleaved into that loop so the PE stays busy while rw streams in.

Sharding: 8 cores = 2 batches x 4 chunks of 576 p-columns (padded to 640 =
5 stages of 128).  Kernel 2 (per core = batch x row-quarter): 4 dilated
fuse convs on a 40-row halo slab.  Host does im2col / col2im / packing.
"""
import math
import numpy as np
import ml_dtypes
from contextlib import ExitStack

import concourse.bass as bass
import concourse.bacc as bacc
import concourse.tile as tile
import concourse.mybir as mybir
from concourse import bass_utils
from concourse.bass import ts
from concourse.masks import make_identity

BF16 = mybir.dt.bfloat16
F8 = mybir.dt.float8e4
F32 = mybir.dt.float32
H = W = 48
L = H * W           # 2304
C = 128
CHUNK = 576         # L/4 p-columns per core
CHUNKP = 640        # padded to 5*128
SCALE = 10.0
DILS = (1, 2, 4, 8)
TS = 416            # l-tile for mm1 softmax slices (4*416 = 1664)
F8S = 32.0          # fp8 rebalance: lhs/F8S, rhs*F8S

_cache = {}


# ---------------------------------------------------------------- host prep
def _im2col3(x):
    # x [C,H,W] -> [C*9, H*W] with zero pad 1 (c-major, then ki, kj)
    Cc, Hh, Ww = x.shape
    xp = np.pad(x, ((0, 0), (1, 1), (1, 1)))
    cols = np.empty((Cc, 3, 3, Hh, Ww), np.float32)
    for ki in range(3):
        for kj in range(3):
            cols[:, ki, kj] = xp[:, ki:ki + Hh, kj:kj + Ww]
    return cols.reshape(Cc * 9, Hh * Ww)


def _im2col4s2(x):
    # x [C,96,96] -> [L, C*16], k=4 stride 2 pad 1
    Cc = x.shape[0]
    xp = np.pad(x, ((0, 0), (1, 1), (1, 1)))
    out = np.empty((H, W, Cc, 4, 4), np.float32)
    for ki in range(4):
        for kj in range(4):
            out[:, :, :, ki, kj] = xp[:, ki:ki + 2 * H:2, kj:kj + 2 * W:2].transpose(1, 2, 0)
    return out.reshape(L, Cc * 16)


def _neighbor_mask():
    M = np.zeros((L, L), np.float32)
    p = np.arange(L)
    pi, pj = p // W, p % W
    for off, sel in ((-1, pj >= 1), (1, pj <= W - 2), (W, pi <= H - 2), (-W, pi >= 1)):
        M[p[sel] + off, p[sel]] = 1.0
    return M


def _col2im(col):
    # col [L, C*16] -> [C, 96, 96] scatter-add (stride 2, pad 1)
    colr = col.reshape(H, W, C, 4, 4)
    out = np.zeros((C, 99, 99), np.float32)
    for ki in range(4):
        for kj in range(4):
            out[:, ki:ki + 96:2, kj:kj + 96:2] += colr[:, :, :, ki, kj].transpose(2, 0, 1)
    return out[:, 1:97, 1:97]


def _pack_part(a, p):
    # [N, F] -> [p, N//p, F] partition-major packing (row r = t*p + pp)
    n, f = a.shape
    return np.ascontiguousarray(a.reshape(n // p, p, f).transpose(1, 0, 2))


def _slices(nlt):
    # l-slices (multiples of 128, each <= 512) shared by host packing and
    # the device build
    NLP = nlt * 128
    nY = (NLP + 511) // 512
    ktps = [nlt // nY + (1 if i < nlt % nY else 0) for i in range(nY)]
    SL = []
    off = 0
    for g in ktps:
        SL.append((off, g * 128))
        off += g * 128
    return nY, ktps, SL


# ---------------------------------------------------------------- kernel 1
def _build_main(nlt, nf8):
    """Gram matmul -> softmax -> transpose -> second matmul, pipelined.

    nlt: number of 128-row l-tiles after compaction (13 for the 24x24 hole).
    nf8: number of fp8 p-stages (0..3); stages nf8..4 are bf16.
    """
    NLP = nlt * 128
    nbf = 5 - nf8
    cinv = float(L - NLP)          # analytic denominator correction count
    nk8 = 10                       # fp8 k-tiles (1152 padded to 1280)

    # l-slices for mm1/softmax: nY PSUM banks, each <= 512 wide.
    nY, ktps, SL = _slices(nlt)
    nP2 = min(4, 8 - nY - 1)       # mm2 PSUM banks (+1 bank for transposes)

    nc = bacc.Bacc("TRN2", target_bir_lowering=False, debug=False, num_devices=8)
    rhsb = nc.dram_tensor("rhsb", [128, 9, NLP], BF16, kind="ExternalInput").ap()
    lhsb = nc.dram_tensor("lhsb", [128, nbf, 9, 128], BF16, kind="ExternalInput").ap()
    post = nc.dram_tensor("post", [128, 5, NLP], BF16, kind="ExternalInput").ap()
    rw = nc.dram_tensor("rw", [128, nlt, 2048], BF16, kind="ExternalInput").ap()
    col = nc.dram_tensor("col", [128, 5, 2048], F32, kind="ExternalOutput").ap()
    if nf8:
        rhs8 = nc.dram_tensor("rhs8", [128, nk8 * NLP], F8, kind="ExternalInput").ap()
        lhs8 = nc.dram_tensor("lhs8", [128, nf8, nk8, 128], F8, kind="ExternalInput").ap()

    with tile.TileContext(nc) as tc, ExitStack() as ctx:
        const = ctx.enter_context(tc.tile_pool(name="const", bufs=1))
        ident = const.tile([128, 128], BF16)
        make_identity(nc, ident)
        zero1 = const.tile([128, 1], F32)
        nc.vector.memset(zero1, 0.0)
        ins = ctx.enter_context(tc.tile_pool(name="ins", bufs=1))
        s_rhsb = ins.tile([128, 9, NLP], BF16, tag="rhsb")
        s_lhsb = ins.tile([128, nbf, 9, 128], BF16, tag="lhsb")
        s_post = ins.tile([128, 5, NLP], BF16, tag="post")
        s_rw = ins.tile([128, nlt, 2048], BF16, tag="rw")
        if nf8:
            s_rhs8 = [ins.tile([128, nk8, sz], F8, tag=f"rhs8_{n}",
                               name=f"s_rhs8_{n}") for n, (o, sz) in enumerate(SL)]
            s_lhs8 = ins.tile([128, nf8, nk8, 128], F8, tag="lhs8")

        # ---- input DMA, first-use order -------------------------------
        o0, s0 = SL[0]
        if nf8:
            nc.sync.dma_start(s_lhs8[:, 0], lhs8[:, 0])
            for n, (o, sz) in enumerate(SL):   # slice-major, full-BW copies
                nc.sync.dma_start(s_rhs8[n], rhs8[:, nk8 * o:nk8 * (o + sz)])
            for m in range(1, nf8):
                nc.sync.dma_start(s_lhs8[:, m], lhs8[:, m])
            nc.sync.dma_start(s_post[:, 0], post[:, 0])
            nc.sync.dma_start(s_post[:, 1], post[:, 1])
            for k in range(nlt):
                nc.sync.dma_start(s_rw[:, k], rw[:, k])
            nc.sync.dma_start(s_lhsb, lhsb)
            for o, s in SL:
                nc.sync.dma_start(s_rhsb[:, :, o:o + s], rhsb[:, :, o:o + s])
            for m in range(2, 5):
                nc.sync.dma_start(s_post[:, m], post[:, m])
        else:
            nc.sync.dma_start(s_lhsb[:, 0], lhsb[:, 0])
            for k in range(9):
                nc.sync.dma_start(s_rhsb[:, k, o0:o0 + s0], rhsb[:, k, o0:o0 + s0])
            for o, s in SL[1:]:
                nc.sync.dma_start(s_rhsb[:, :, o:o + s], rhsb[:, :, o:o + s])
            for m in range(1, 5):
                nc.sync.dma_start(s_lhsb[:, m], lhsb[:, m])
            nc.sync.dma_start(s_post[:, 0], post[:, 0])
            for k in range(nlt):
                nc.sync.dma_start(s_rw[:, k], rw[:, k])
            for m in range(1, 5):
                nc.sync.dma_start(s_post[:, m], post[:, m])

        # PSUM: nY banks mm1 accum (Y) + nP2 banks mm2 (P2) + 1 transposes
        psumY = ctx.enter_context(tc.tile_pool(name="psumY", bufs=nY, space="PSUM"))
        psumP = ctx.enter_context(tc.tile_pool(name="psumP", bufs=1, space="PSUM"))
        work = ctx.enter_context(tc.tile_pool(name="work", bufs=2))
        stats = ctx.enter_context(tc.tile_pool(name="stats", bufs=3))

        stage = {}   # m -> (Ys, negm)

        def mm1_gen(m):
            """Generator: emits stage-m Gram matmuls one at a time."""
            mx = stats.tile([128, nY], F32, tag="mx")
            Ys = []
            for n, (off, sz) in enumerate(SL):
                Y = psumY.tile([128, 512], F32, tag="Y")
                if m < nf8:
                    for kp in range(nk8 // 2):
                        nc.tensor.matmul(
                            Y[:, :sz],
                            lhsT=s_lhs8[:, m, 2 * kp:2 * kp + 2, :],
                            rhs=s_rhs8[n][:, 2 * kp:2 * kp + 2, :],
                            start=(kp == 0), stop=(kp == nk8 // 2 - 1),
                            perf_mode=mybir.MatmulPerfMode.DoubleRow,
                        )
                        yield
                else:
                    for k in range(9):
                        nc.tensor.matmul(
                            Y[:, :sz],
                            lhsT=s_lhsb[:, m - nf8, k, :],
                            rhs=s_rhsb[:, k, off:off + sz],
                            start=(k == 0), stop=(k == 8),
                        )
                        yield
                nc.vector.reduce_max(mx[:, n:n + 1], Y[:, :sz],
                                     axis=mybir.AxisListType.X)
                Ys.append(Y)
            negm = stats.tile([128, 1], F32, tag="negm")
            nc.vector.tensor_reduce(negm, mx, axis=mybir.AxisListType.X,
                                    op=mybir.AluOpType.max, negate=True)
            stage[m] = (Ys, negm)

        def chain(m):
            """Softmax chain for stage m: Y -> yt (bf16), rcp.  Act + DVE.
            The 1e-8 floor is folded into a host-side correction (yt keeps
            sm*post un-floored; host adds 1e-8*RW.sum(0) to col)."""
            Ys, negm = stage.pop(m)
            sums = stats.tile([128, nY], F32, tag="sums")
            yt = work.tile([128, NLP], BF16, tag="yt")
            for n, (off, sz) in enumerate(SL):
                ye = work.tile([128, 512], BF16, tag="ye")
                nc.scalar.activation(ye[:, :sz], Ys[n][:, :sz],
                                     mybir.ActivationFunctionType.Exp,
                                     bias=negm, accum_out=sums[:, n:n + 1])
                nc.vector.tensor_mul(yt[:, off:off + sz], ye[:, :sz],
                                     s_post[:, m, off:off + sz])
            stot = stats.tile([128, 1], F32, tag="stot")
            nc.vector.reduce_sum(stot, sums, axis=mybir.AxisListType.X)
            stot2 = stats.tile([128, 1], F32, tag="stot2")
            if cinv > 0:
                negmc = stats.tile([128, 1], F32, tag="negmc")
                nc.vector.tensor_scalar_add(negmc, negm, math.log(cinv))
                ec = stats.tile([128, 1], F32, tag="ec")
                nc.scalar.activation(ec, negmc,
                                     mybir.ActivationFunctionType.Exp,
                                     bias=zero1)
                nc.vector.tensor_add(stot2, stot, ec)
            else:
                nc.vector.tensor_copy(stot2, stot)
            rcp = stats.tile([128, 1], F32, tag="rcp")
            nc.vector.reciprocal(rcp, stot2)
            return yt, rcp

        def rest(m, yt, rcp, chain_next=None, fill=None, fill_delay=0,
                 big_first=False, chain_k=None):
            """Transposes + mm2 for stage m.  mm2 runs as two bank-groups
            (sizes nP2 and 4-nP2); chain(m+1) is emitted either at k-index
            chain_k of the big group (DMA-paced stage 0) or right after the
            small group so its act/DVE work overlaps the big group's k-loop.
            mm1(m+2) matmuls are interleaved into the big group only."""
            ytT = work.tile([128, nlt, 128], BF16, tag="ytT")

            def t_batch(n, t0, g):   # T batch n only needs yt slice n
                pT = psumP.tile([128, 4, 128], BF16, tag="pT", bufs=1,
                                name=f"pT_{m}_{n}")
                for k in range(t0, t0 + g):
                    nc.tensor.transpose(pT[:, k - t0, :], yt[:, ts(k, 128)], ident)
                nc.vector.tensor_copy(ytT[:, t0:t0 + g], pT[:, :g])

            # remaining batches triggered inside the first k-loop, 2 k-steps
            # before their tiles are consumed (hides the PSUM evict)
            trig = {}
            t0 = ktps[0]
            for n, g in enumerate(ktps[1:], 1):
                trig[max(0, t0 - 2)] = (n, t0, g)
                t0 += g
            t_batch(0, 0, ktps[0])
            nxt = [None]
            colm = work.tile([128, 2048], F32, tag="colm")
            frate = 2 if fill_delay else 3
            big = list(range(nP2))
            small = list(range(nP2, 4))
            groups = [big, small] if big_first else [small, big]

            def do_chain():
                if chain_next:
                    nxt[0] = chain_next()

            for gi, grp in enumerate(groups):
                P2 = {nn: psumP.tile([128, 512], F32, tag="P2", bufs=nP2,
                                     name=f"P2_{m}_{nn}") for nn in grp}
                isbig = len(grp) == nP2
                for k in range(nlt):
                    if gi == 0 and k in trig:
                        t_batch(*trig[k])
                    if isbig and k == chain_k:
                        do_chain()
                    for nn in grp:
                        nc.tensor.matmul(P2[nn], lhsT=ytT[:, k, :],
                                         rhs=s_rw[:, k, ts(nn, 512)],
                                         start=(k == 0), stop=(k == nlt - 1))
                    if fill is not None and isbig and k >= fill_delay:
                        for _ in range(frate):
                            if next(fill, "end") == "end":
                                fill = None
                                break
                for nn in grp:
                    nc.scalar.activation(colm[:, ts(nn, 512)], P2[nn],
                                         mybir.ActivationFunctionType.Copy,
                                         scale=rcp)
                    nc.sync.dma_start(col[:, m, ts(nn, 512)],
                                      colm[:, ts(nn, 512)])
                if gi == 0 and chain_k is None:
                    do_chain()
            if fill is not None:
                for _ in fill:
                    pass
            return nxt[0]

        # ---- software pipeline ---------------------------------------
        for _ in mm1_gen(0):
            pass
        for _ in mm1_gen(1):
            pass
        cur = chain(0)          # yt(0), rcp(0)
        for m in range(5):
            cn = (lambda mm=m + 1: chain(mm)) if m + 1 < 5 else None
            fl = mm1_gen(m + 2) if m + 2 < 5 else None
            # bf16 fills wait on the late rhsb DMA -> delay so an unmet
            # dep doesn't block the in-order PE queue mid-k-loop
            fd = 4 if (fl is not None and m + 2 >= nf8) else 0
            cur = rest(m, *cur, chain_next=cn, fill=fl, fill_delay=fd,
                       big_first=(m == 0 or m == 4),
                       chain_k=8 if m == 0 else None)
    nc.compile()
    return nc


# ---------------------------------------------------------------- kernel 2
def _build_fuse():
    """4 dilated 3x3 convs.  Groups are packed at 32-partition strides of a
    single [128, n] PSUM tile: the shared center tap becomes ONE M=128
    matmul, and relu+bias is one full-partition activation per row-tile."""
    nc = bacc.Bacc("TRN2", target_bir_lowering=False, debug=False, num_devices=8)
    y = nc.dram_tensor("yslab", [128, 40, 112], BF16, kind="ExternalInput").ap()
    fw = nc.dram_tensor("fw", [128, 4, 9, 16], BF16, kind="ExternalInput").ap()
    fwc = nc.dram_tensor("fwc", [128, 128], BF16, kind="ExternalInput").ap()
    fbx = nc.dram_tensor("fbx", [128, 1], F32, kind="ExternalInput").ap()
    fo = nc.dram_tensor("fo", [128, 5, 512], F32, kind="ExternalOutput").ap()

    RT = [(0, 5), (5, 5), (10, 5), (15, 5), (20, 4)]
    TAPS = [(ki, kj) for ki in range(3) for kj in range(3) if (ki, kj) != (1, 1)]
    with tile.TileContext(nc) as tc, ExitStack() as ctx:
        const = ctx.enter_context(tc.tile_pool(name="const", bufs=1))
        ident = const.tile([128, 128], BF16)
        make_identity(nc, ident)
        ins = ctx.enter_context(tc.tile_pool(name="ins", bufs=1))
        s_y = ins.tile([128, 40, 112], BF16, tag="y")
        s_w = ins.tile([128, 4, 9, 16], BF16, tag="w")
        s_wc = ins.tile([128, 128], BF16, tag="wc")
        s_b = ins.tile([128, 1], F32, tag="b")
        for rc in range(2):
            nc.sync.dma_start(s_y[:, 8 * rc:8 * rc + 8, :], y[:, 8 * rc:8 * rc + 8, :])
        nc.sync.dma_start(s_wc, fwc)
        nc.sync.dma_start(s_w, fw)
        nc.sync.dma_start(s_y[:, 16:24, :], y[:, 16:24, :])
        nc.sync.dma_start(s_b, fbx)
        for rc in range(3, 5):
            nc.sync.dma_start(s_y[:, 8 * rc:8 * rc + 8, :], y[:, 8 * rc:8 * rc + 8, :])
        psum = ctx.enter_context(tc.tile_pool(name="psum", bufs=4, space="PSUM"))
        work = ctx.enter_context(tc.tile_pool(name="work", bufs=4))
        # p-state warmup: dep-free matmuls keep the PE busy through the
        # input DMA window so real matmuls start at full clock
        warm = psum.tile([128, 512], F32, tag="ps")
        for i in range(8):
            nc.tensor.matmul(warm, lhsT=ident, rhs=ident.to_broadcast(
                [128, 512]) if False else ident, start=(i == 0), stop=(i == 7))
        for r0, nr in RT:
            ps = psum.tile([128, 512], F32, tag="ps")
            n = nr * 96
            nc.tensor.matmul(ps[:, :n], lhsT=s_wc,
                             rhs=s_y[:, 8 + r0:8 + r0 + nr, 8:8 + 96],
                             start=True, stop=False)
            for g in range(4):
                d = DILS[g]
                for i, (ki, kj) in enumerate(TAPS):
                    u0 = 8 + r0 + d * (ki - 1)
                    v0 = 8 + d * (kj - 1)
                    nc.tensor.matmul(
                        ps[32 * g:32 * g + 16, :n],
                        lhsT=s_w[:, g, ki * 3 + kj, :],
                        rhs=s_y[:, u0:u0 + nr, v0:v0 + 96],
                        start=False, stop=(i == 7),
                        tile_position=(0, 32 * g),
                    )
            ob = work.tile([128, 512], F32, tag="ob")
            nc.scalar.activation(ob[:, :n], ps[:, :n],
                                 mybir.ActivationFunctionType.Relu,
                                 bias=s_b)
            nc.sync.dma_start(fo[:, RT.index((r0, nr)), :n], ob[:, :n])
    nc.compile()
    return nc


def _get(name, builder):
    if name not in _cache:
        _cache[name] = builder()
    return _cache[name]


# ---------------------------------------------------------------- entry
def kernel(x1, x2, mask, mask_all, fuse_w, fuse_b, _collect=None):
    x1 = np.asarray(x1, np.float32)
    x2 = np.asarray(x2, np.float32)
    mask = np.asarray(mask, np.float32)
    mask_all = np.asarray(mask_all, np.float32)
    fuse_w = np.asarray(fuse_w, np.float32)
    fuse_b = np.asarray(fuse_b, np.float32)
    N = x1.shape[0]
    bf = ml_dtypes.bfloat16
    f8 = ml_dtypes.float8_e4m3fn

    NB = _neighbor_mask()  # [L, L]

    # ---- per-batch compaction / permutation planning
    plans = []
    nlt = 0
    nf8 = 3
    for b in range(N):
        mp = _im2col3(mask[b])
        mmv = (mp.mean(0) == 0.0).astype(np.float32)
        valid = np.where(mmv > 0)[0]
        nlt = max(nlt, (len(valid) + 127) // 128)
        perms = []
        for j in range(4):
            psel = np.arange(j * CHUNK, (j + 1) * CHUNK)
            nh = psel[mmv[psel] > 0]
            hl = psel[mmv[psel] == 0]
            perms.append((nh, hl))
            if len(nh) < 3 * 128 or len(nh) - 3 * 128 + len(hl) > 2 * 128:
                nf8 = 0
        plans.append((mmv, valid, perms))
    NLP = nlt * 128

    in_maps = []
    unperms = []
    corrs = []
    for b in range(N):
        mmv, valid, perms = plans[b]
        P = _im2col3(x2[b])
        norms = np.sqrt((P * P).sum(0))
        ma = mask_all[b, 0].reshape(L)
        lhs_full = P * (SCALE * ma)[None, :]
        rhs_full = P * (mmv / np.maximum(norms, 1e-4))[None, :]
        rhs_c = np.zeros((1152, NLP), np.float32)
        rhs_c[:, :len(valid)] = rhs_full[:, valid]
        rhsb_r = _pack_part(rhs_c, 128).astype(bf)                # [128,9,NLP]
        if nf8:
            rhs8_c = np.zeros((1280, NLP), np.float32)
            rhs8_c[:1152] = rhs_c * F8S
            r8p = _pack_part(rhs8_c, 128).astype(f8)              # [128,10,NLP]
            _, _, SLh = _slices(nlt)
            rhs8_r = np.concatenate(                              # slice-major
                [r8p[:, :, o:o + s].reshape(128, -1) for o, s in SLh], axis=1)
            rhs8_r = np.ascontiguousarray(rhs8_r)                 # [128,10*NLP]
        RW = _im2col4s2(x1[b])                                    # [L, 2048]
        rw_c = np.zeros((NLP, 2048), np.float32)
        rw_c[:len(valid)] = RW[valid]
        rw_r = _pack_part(rw_c, 128).astype(bf)                   # [128,nlt,2048]
        # 1e-8 floor for ALL l (valid via floor-fold, invalid exactly)
        corrs.append(1e-8 * RW.sum(0))
        postF = (1.0 + 0.5 * NB) * mmv[:, None] * ma[None, :]     # [l, p]
        postC = postF[valid]                                      # [nv, p]
        for j in range(4):
            nh, hl = perms[j]
            if nf8:
                perm = np.concatenate([nh, hl])
            else:
                perm = np.concatenate([nh, hl])
            unperms.append(perm)
            lhs_p = lhs_full[:, perm]                             # [1152, 576]
            post_p = np.zeros((CHUNKP, NLP), np.float32)
            post_p[:CHUNK, :len(valid)] = postC[:, perm].T
            im = {
                "rhsb": rhsb_r,
                "post": _pack_part(post_p, 128).astype(bf),
                "rw": rw_r,
            }
            if nf8:
                im["rhs8"] = rhs8_r
                lf8 = np.zeros((1280, 3 * 128), np.float32)
                lf8[:1152] = lhs_p[:, :3 * 128] / F8S
                l8 = _pack_part(lf8, 128).reshape(128, 10, 3, 128)
                im["lhs8"] = np.ascontiguousarray(
                    l8.transpose(0, 2, 1, 3)).astype(f8)          # [128,3,10,128]
                lbf = np.zeros((1152, 2 * 128), np.float32)
                lbf[:, :CHUNK - 3 * 128] = lhs_p[:, 3 * 128:]
                lb = _pack_part(lbf, 128).reshape(128, 9, 2, 128)
                im["lhsb"] = np.ascontiguousarray(
                    lb.transpose(0, 2, 1, 3)).astype(bf)          # [128,2,9,128]
            else:
                lbf = np.zeros((1152, CHUNKP), np.float32)
                lbf[:, :CHUNK] = lhs_p
                lb = _pack_part(lbf, 128).reshape(128, 9, 5, 128)
                im["lhsb"] = np.ascontiguousarray(
                    lb.transpose(0, 2, 1, 3)).astype(bf)          # [128,5,9,128]
            in_maps.append(im)

    nc1 = _get(f"main_{nlt}_{nf8}", lambda: _build_main(nlt, nf8))
    res1 = bass_utils.run_bass_kernel_spmd(nc1, in_maps, core_ids=list(range(8)))
    if _collect is not None:
        _collect.append(res1)

    ys = []
    for b in range(N):
        col = np.empty((L, 2048), np.float32)
        for j in range(4):
            r = res1.results[b * 4 + j]["col"]     # [128, 5, 2048]
            colP = r.transpose(1, 0, 2).reshape(CHUNKP, 2048)[:CHUNK]
            col[unperms[b * 4 + j]] = colP
        col += corrs[b][None, :]
        ys.append(_col2im(col) / 4.0)
    y = np.stack(ys)                               # [N, 128, 96, 96]

    fw_r = np.ascontiguousarray(
        fuse_w.transpose(2, 0, 3, 4, 1).reshape(128, 4, 9, 16)).astype(bf)
    fwc_r = np.zeros((128, 128), np.float32)       # center taps, 32-stride
    fbx_r = np.zeros((128, 1), np.float32)
    for g in range(4):
        fwc_r[:, 32 * g:32 * g + 16] = fuse_w[g, :, :, 1, 1].T
        fbx_r[32 * g:32 * g + 16, 0] = fuse_b[g]
    fwc_r = fwc_r.astype(bf)
    in_maps2 = []
    for b in range(N):
        yp = np.pad(y[b], ((0, 0), (8, 8), (8, 8))).astype(bf)  # [128,112,112]
        for q in range(4):
            in_maps2.append({
                "yslab": np.ascontiguousarray(yp[:, 24 * q:24 * q + 40, :]),
                "fw": fw_r, "fwc": fwc_r, "fbx": fbx_r,
            })
    nc2 = _get("fuse", _build_fuse)
    res2 = bass_utils.run_bass_kernel_spmd(nc2, in_maps2, core_ids=list(range(8)))
    if _collect is not None:
        _collect.append(res2)

    out = np.empty((N, 64, 96, 96), np.float32)
    for b in range(N):
        for q in range(4):
            r = res2.results[b * 4 + q]            # [16, 4, 2304]
            o = r["fo"].reshape(16, 4, 24, 96)
            out[b, :, 24 * q:24 * q + 24, :] = o.transpose(1, 0, 2, 3).reshape(64, 24, 96)
    return out


# revision 34
# speedup vs baseline: 1.4580x; 1.1390x over previous
"""AtnConv (contextual attention) Trainium2 Bass kernel, 8-core SPMD.

Decomposition (per batch b, L=2304=48*48 patches, C=128):
  P  = im2col3x3(x2_pad)                    [1152, L]
  logits[p, l] = (P[:,p]*10*ma[p]) . (P[:,l]*mm[l]/max(|P[:,l]|,1e-4))
  sm = softmax over l (free dim)            [p, l]
  Yt = max(sm * post[l,p], 1e-8),  post = (1+0.5*mask_c)*mm[l]*ma[p]
  col[p, :] = Yt @ RW,  RW = im2col4x4s2(x1_pad)  [L, 2048]
  y = col2im(col)/4 ; out = concat_g relu(dilated_conv3x3(y, fuse_w[g]) + fuse_b[g])

Key optimizations over the naive mapping:
  * l-compaction: columns l with mm[l]=0 (hole patches) have logits == 0
    exactly (the mask is folded into the rhs scaling), so they are dropped
    from the Gram matmul and from the second matmul's contraction.  Their
    softmax-denominator contribution is exp(-max) each, added analytically
    on-device; their 1e-8*RW[l] contribution to col is a p-independent
    vector added on host.
  * fp8 (e4m3, DoubleRow) Gram matmul for p-rows whose self-match column is
    valid: the softmax there is dominated by the exact self-match logit
    (~34x larger than cross terms), so fp8 logit noise is invisible in the
    output.  Hole p-rows (soft attention) stay bf16.  Host permutes p so
    fp8/bf16 rows land in separate 128-col stages.
  * mm2 runs on 4 PSUM banks in a single k-loop; the next-next stage's mm1
    is interleaved into that loop so the PE stays busy while rw streams in.

Sharding: 8 cores = 2 batches x 4 chunks of 576 p-columns (padded to 640 =
5 stages of 128).  Kernel 2 (per core = batch x row-quarter): 4 dilated
fuse convs on a 40-row halo slab.  Host does im2col / col2im / packing.
"""
import math
import numpy as np
import ml_dtypes
from contextlib import ExitStack

import concourse.bass as bass
import concourse.bacc as bacc
import concourse.tile as tile
import concourse.mybir as mybir
from concourse import bass_utils
from concourse.bass import ts
from concourse.masks import make_identity

BF16 = mybir.dt.bfloat16
F8 = mybir.dt.float8e4
F32 = mybir.dt.float32
H = W = 48
L = H * W           # 2304
C = 128
CHUNK = 576         # L/4 p-columns per core
CHUNKP = 640        # padded to 5*128
SCALE = 10.0
DILS = (1, 2, 4, 8)
TS = 416            # l-tile for mm1 softmax slices (4*416 = 1664)
F8S = 32.0          # fp8 rebalance: lhs/F8S, rhs*F8S

_cache = {}


# ---------------------------------------------------------------- host prep
def _im2col3(x):
    # x [C,H,W] -> [C*9, H*W] with zero pad 1 (c-major, then ki, kj)
    Cc, Hh, Ww = x.shape
    xp = np.pad(x, ((0, 0), (1, 1), (1, 1)))
    cols = np.empty((Cc, 3, 3, Hh, Ww), np.float32)
    for ki in range(3):
        for kj in range(3):
            cols[:, ki, kj] = xp[:, ki:ki + Hh, kj:kj + Ww]
    return cols.reshape(Cc * 9, Hh * Ww)


def _im2col4s2(x):
    # x [C,96,96] -> [L, C*16], k=4 stride 2 pad 1
    Cc = x.shape[0]
    xp = np.pad(x, ((0, 0), (1, 1), (1, 1)))
    out = np.empty((H, W, Cc, 4, 4), np.float32)
    for ki in range(4):
        for kj in range(4):
            out[:, :, :, ki, kj] = xp[:, ki:ki + 2 * H:2, kj:kj + 2 * W:2].transpose(1, 2, 0)
    return out.reshape(L, Cc * 16)


def _neighbor_mask():
    M = np.zeros((L, L), np.float32)
    p = np.arange(L)
    pi, pj = p // W, p % W
    for off, sel in ((-1, pj >= 1), (1, pj <= W - 2), (W, pi <= H - 2), (-W, pi >= 1)):
        M[p[sel] + off, p[sel]] = 1.0
    return M


def _col2im(col):
    # col [L, C*16] -> [C, 96, 96] scatter-add (stride 2, pad 1)
    colr = col.reshape(H, W, C, 4, 4)
    out = np.zeros((C, 99, 99), np.float32)
    for ki in range(4):
        for kj in range(4):
            out[:, ki:ki + 96:2, kj:kj + 96:2] += colr[:, :, :, ki, kj].transpose(2, 0, 1)
    return out[:, 1:97, 1:97]


def _pack_part(a, p):
    # [N, F] -> [p, N//p, F] partition-major packing (row r = t*p + pp)
    n, f = a.shape
    return np.ascontiguousarray(a.reshape(n // p, p, f).transpose(1, 0, 2))


def _slices(nlt):
    # l-slices (multiples of 128, each <= 512) shared by host packing and
    # the device build
    NLP = nlt * 128
    nY = (NLP + 511) // 512
    ktps = [nlt // nY + (1 if i < nlt % nY else 0) for i in range(nY)]
    SL = []
    off = 0
    for g in ktps:
        SL.append((off, g * 128))
        off += g * 128
    return nY, ktps, SL


# ---------------------------------------------------------------- kernel 1
def _build_main(nlt, nf8):
    """Gram matmul -> softmax -> transpose -> second matmul, pipelined.

    nlt: number of 128-row l-tiles after compaction (13 for the 24x24 hole).
    nf8: number of fp8 p-stages (0..3); stages nf8..4 are bf16.
    """
    NLP = nlt * 128
    nbf = 5 - nf8
    cinv = float(L - NLP)          # analytic denominator correction count
    nk8 = 10                       # fp8 k-tiles (1152 padded to 1280)

    # l-slices for mm1/softmax: nY PSUM banks, each <= 512 wide.
    nY, ktps, SL = _slices(nlt)
    nP2 = min(4, 8 - nY - 1)       # mm2 PSUM banks (+1 bank for transposes)

    nc = bacc.Bacc("TRN2", target_bir_lowering=False, debug=False, num_devices=8)
    rhsb = nc.dram_tensor("rhsb", [128, 9, NLP], BF16, kind="ExternalInput").ap()
    lhsb = nc.dram_tensor("lhsb", [128, nbf, 9, 128], BF16, kind="ExternalInput").ap()
    post = nc.dram_tensor("post", [128, 5, NLP], BF16, kind="ExternalInput").ap()
    rw = nc.dram_tensor("rw", [128, nlt, 2048], BF16, kind="ExternalInput").ap()
    col = nc.dram_tensor("col", [128, 5, 2048], F32, kind="ExternalOutput").ap()
    if nf8:
        rhs8 = nc.dram_tensor("rhs8", [128, nk8 * NLP], F8, kind="ExternalInput").ap()
        lhs8 = nc.dram_tensor("lhs8", [128, nf8, nk8, 128], F8, kind="ExternalInput").ap()

    with tile.TileContext(nc) as tc, ExitStack() as ctx:
        const = ctx.enter_context(tc.tile_pool(name="const", bufs=1))
        ident = const.tile([128, 128], BF16)
        make_identity(nc, ident)
        zero1 = const.tile([128, 1], F32)
        nc.vector.memset(zero1, 0.0)
        ins = ctx.enter_context(tc.tile_pool(name="ins", bufs=1))
        s_rhsb = ins.tile([128, 9, NLP], BF16, tag="rhsb")
        s_lhsb = ins.tile([128, nbf, 9, 128], BF16, tag="lhsb")
        s_post = ins.tile([128, 5, NLP], BF16, tag="post")
        s_rw = ins.tile([128, nlt, 2048], BF16, tag="rw")
        if nf8:
            s_rhs8 = [ins.tile([128, nk8, sz], F8, tag=f"rhs8_{n}",
                               name=f"s_rhs8_{n}") for n, (o, sz) in enumerate(SL)]
            s_lhs8 = ins.tile([128, nf8, nk8, 128], F8, tag="lhs8")

        # ---- input DMA, first-use order -------------------------------
        o0, s0 = SL[0]
        if nf8:
            nc.sync.dma_start(s_lhs8[:, 0], lhs8[:, 0])
            for n, (o, sz) in enumerate(SL):   # slice-major, full-BW copies
                nc.sync.dma_start(s_rhs8[n], rhs8[:, nk8 * o:nk8 * (o + sz)])
            for m in range(1, nf8):
                nc.sync.dma_start(s_lhs8[:, m], lhs8[:, m])
            nc.sync.dma_start(s_post[:, 0], post[:, 0])
            nc.sync.dma_start(s_post[:, 1], post[:, 1])
            for k in range(nlt):
                nc.sync.dma_start(s_rw[:, k], rw[:, k])
            nc.sync.dma_start(s_lhsb, lhsb)
            for o, s in SL:
                nc.sync.dma_start(s_rhsb[:, :, o:o + s], rhsb[:, :, o:o + s])
            for m in range(2, 5):
                nc.sync.dma_start(s_post[:, m], post[:, m])
        else:
            nc.sync.dma_start(s_lhsb[:, 0], lhsb[:, 0])
            for k in range(9):
                nc.sync.dma_start(s_rhsb[:, k, o0:o0 + s0], rhsb[:, k, o0:o0 + s0])
            for o, s in SL[1:]:
                nc.sync.dma_start(s_rhsb[:, :, o:o + s], rhsb[:, :, o:o + s])
            for m in range(1, 5):
                nc.sync.dma_start(s_lhsb[:, m], lhsb[:, m])
            nc.sync.dma_start(s_post[:, 0], post[:, 0])
            for k in range(nlt):
                nc.sync.dma_start(s_rw[:, k], rw[:, k])
            for m in range(1, 5):
                nc.sync.dma_start(s_post[:, m], post[:, m])

        # PSUM: nY banks mm1 accum (Y) + nP2 banks mm2 (P2) + 1 transposes
        psumY = ctx.enter_context(tc.tile_pool(name="psumY", bufs=nY, space="PSUM"))
        psumP = ctx.enter_context(tc.tile_pool(name="psumP", bufs=1, space="PSUM"))
        work = ctx.enter_context(tc.tile_pool(name="work", bufs=2))
        stats = ctx.enter_context(tc.tile_pool(name="stats", bufs=3))

        stage = {}   # m -> (Ys, negm)

        def mm1_gen(m):
            """Generator: emits stage-m Gram matmuls one at a time."""
            mx = stats.tile([128, nY], F32, tag="mx")
            Ys = []
            for n, (off, sz) in enumerate(SL):
                Y = psumY.tile([128, 512], F32, tag="Y")
                if m < nf8:
                    for kp in range(nk8 // 2):
                        nc.tensor.matmul(
                            Y[:, :sz],
                            lhsT=s_lhs8[:, m, 2 * kp:2 * kp + 2, :],
                            rhs=s_rhs8[n][:, 2 * kp:2 * kp + 2, :],
                            start=(kp == 0), stop=(kp == nk8 // 2 - 1),
                            perf_mode=mybir.MatmulPerfMode.DoubleRow,
                        )
                        yield
                else:
                    for k in range(9):
                        nc.tensor.matmul(
                            Y[:, :sz],
                            lhsT=s_lhsb[:, m - nf8, k, :],
                            rhs=s_rhsb[:, k, off:off + sz],
                            start=(k == 0), stop=(k == 8),
                        )
                        yield
                nc.vector.reduce_max(mx[:, n:n + 1], Y[:, :sz],
                                     axis=mybir.AxisListType.X)
                Ys.append(Y)
            negm = stats.tile([128, 1], F32, tag="negm")
            nc.vector.tensor_reduce(negm, mx, axis=mybir.AxisListType.X,
                                    op=mybir.AluOpType.max, negate=True)
            stage[m] = (Ys, negm)

        def chain(m):
            """Softmax chain for stage m: Y -> yt (bf16), rcp.  Act + DVE.
            The 1e-8 floor is folded into a host-side correction (yt keeps
            sm*post un-floored; host adds 1e-8*RW.sum(0) to col)."""
            Ys, negm = stage.pop(m)
            sums = stats.tile([128, nY], F32, tag="sums")
            yt = work.tile([128, NLP], BF16, tag="yt")
            for n, (off, sz) in enumerate(SL):
                ye = work.tile([128, 512], BF16, tag="ye")
                nc.scalar.activation(ye[:, :sz], Ys[n][:, :sz],
                                     mybir.ActivationFunctionType.Exp,
                                     bias=negm, accum_out=sums[:, n:n + 1])
                nc.vector.tensor_mul(yt[:, off:off + sz], ye[:, :sz],
                                     s_post[:, m, off:off + sz])
            stot = stats.tile([128, 1], F32, tag="stot")
            nc.vector.reduce_sum(stot, sums, axis=mybir.AxisListType.X)
            stot2 = stats.tile([128, 1], F32, tag="stot2")
            if cinv > 0:
                negmc = stats.tile([128, 1], F32, tag="negmc")
                nc.vector.tensor_scalar_add(negmc, negm, math.log(cinv))
                ec = stats.tile([128, 1], F32, tag="ec")
                nc.scalar.activation(ec, negmc,
                                     mybir.ActivationFunctionType.Exp,
                                     bias=zero1)
                nc.vector.tensor_add(stot2, stot, ec)
            else:
                nc.vector.tensor_copy(stot2, stot)
            rcp = stats.tile([128, 1], F32, tag="rcp")
            nc.vector.reciprocal(rcp, stot2)
            return yt, rcp

        def rest(m, yt, rcp, chain_next=None, fill=None, fill_delay=0,
                 big_first=False, chain_k=None):
            """Transposes + mm2 for stage m.  mm2 runs as two bank-groups
            (sizes nP2 and 4-nP2); chain(m+1) is emitted either at k-index
            chain_k of the big group (DMA-paced stage 0) or right after the
            small group so its act/DVE work overlaps the big group's k-loop.
            mm1(m+2) matmuls are interleaved into the big group only."""
            ytT = work.tile([128, nlt, 128], BF16, tag="ytT")

            def t_batch(n, t0, g):   # T batch n only needs yt slice n
                pT = psumP.tile([128, 4, 128], BF16, tag="pT", bufs=1,
                                name=f"pT_{m}_{n}")
                for k in range(t0, t0 + g):
                    nc.tensor.transpose(pT[:, k - t0, :], yt[:, ts(k, 128)], ident)
                nc.vector.tensor_copy(ytT[:, t0:t0 + g], pT[:, :g])

            # remaining batches triggered inside the first k-loop, 2 k-steps
            # before their tiles are consumed (hides the PSUM evict)
            trig = {}
            t0 = ktps[0]
            for n, g in enumerate(ktps[1:], 1):
                trig[max(0, t0 - 2)] = (n, t0, g)
                t0 += g
            t_batch(0, 0, ktps[0])
            nxt = [None]
            colm = work.tile([128, 2048], F32, tag="colm")
            frate = 2 if fill_delay else 3
            big = list(range(nP2))
            small = list(range(nP2, 4))
            groups = [big, small] if big_first else [small, big]

            def do_chain():
                if chain_next:
                    nxt[0] = chain_next()

            for gi, grp in enumerate(groups):
                P2 = {nn: psumP.tile([128, 512], F32, tag="P2", bufs=nP2,
                                     name=f"P2_{m}_{nn}") for nn in grp}
                isbig = len(grp) == nP2
                for k in range(nlt):
                    if gi == 0 and k in trig:
                        t_batch(*trig[k])
                    if isbig and k == chain_k:
                        do_chain()
                    for nn in grp:
                        nc.tensor.matmul(P2[nn], lhsT=ytT[:, k, :],
                                         rhs=s_rw[:, k, ts(nn, 512)],
                                         start=(k == 0), stop=(k == nlt - 1))
                    if fill is not None and isbig and k >= fill_delay:
                        for _ in range(frate):
                            if next(fill, "end") == "end":
                                fill = None
                                break
                for nn in grp:
                    nc.scalar.activation(colm[:, ts(nn, 512)], P2[nn],
                                         mybir.ActivationFunctionType.Copy,
                                         scale=rcp)
                    nc.sync.dma_start(col[:, m, ts(nn, 512)],
                                      colm[:, ts(nn, 512)])
                if gi == 0 and chain_k is None:
                    do_chain()
            if fill is not None:
                for _ in fill:
                    pass
            return nxt[0]

        # ---- software pipeline ---------------------------------------
        for _ in mm1_gen(0):
            pass
        for _ in mm1_gen(1):
            pass
        cur = chain(0)          # yt(0), rcp(0)
        for m in range(5):
            cn = (lambda mm=m + 1: chain(mm)) if m + 1 < 5 else None
            fl = mm1_gen(m + 2) if m + 2 < 5 else None
            # bf16 fills wait on the late rhsb DMA -> delay so an unmet
            # dep doesn't block the in-order PE queue mid-k-loop
            fd = 4 if (fl is not None and m + 2 >= nf8) else 0
            cur = rest(m, *cur, chain_next=cn, fill=fl, fill_delay=fd,
                       big_first=(m == 0 or m == 4),
                       chain_k=8 if m == 0 else None)
    nc.compile()
    return nc


# ---------------------------------------------------------------- kernel 2
def _build_fuse():
    """4 dilated 3x3 convs.  Groups are packed at 32-partition strides of a
    single [128, n] PSUM tile: the shared center tap becomes ONE M=128
    matmul, and relu+bias is one full-partition activation per row-tile."""
    nc = bacc.Bacc("TRN2", target_bir_lowering=False, debug=False, num_devices=8)
    y = nc.dram_tensor("yslab", [128, 40, 112], BF16, kind="ExternalInput").ap()
    fw = nc.dram_tensor("fw", [128, 4, 9, 16], BF16, kind="ExternalInput").ap()
    fwc = nc.dram_tensor("fwc", [128, 128], BF16, kind="ExternalInput").ap()
    fbx = nc.dram_tensor("fbx", [128, 1], F32, kind="ExternalInput").ap()
    fo = nc.dram_tensor("fo", [128, 5, 512], F32, kind="ExternalOutput").ap()

    RT = [(0, 5), (5, 5), (10, 5), (15, 5), (20, 4)]
    TAPS = [(ki, kj) for ki in range(3) for kj in range(3) if (ki, kj) != (1, 1)]
    with tile.TileContext(nc) as tc, ExitStack() as ctx:
        const = ctx.enter_context(tc.tile_pool(name="const", bufs=1))
        ident = const.tile([128, 128], BF16)
        make_identity(nc, ident)
        ins = ctx.enter_context(tc.tile_pool(name="ins", bufs=1))
        s_y = ins.tile([128, 40, 112], BF16, tag="y")
        s_w = ins.tile([128, 4, 9, 16], BF16, tag="w")
        s_wc = ins.tile([128, 128], BF16, tag="wc")
        s_b = ins.tile([128, 1], F32, tag="b")
        for rc in range(2):
            nc.sync.dma_start(s_y[:, 8 * rc:8 * rc + 8, :], y[:, 8 * rc:8 * rc + 8, :])
        nc.sync.dma_start(s_wc, fwc)
        nc.sync.dma_start(s_w, fw)
        nc.sync.dma_start(s_y[:, 16:24, :], y[:, 16:24, :])
        nc.sync.dma_start(s_b, fbx)
        for rc in range(3, 5):
            nc.sync.dma_start(s_y[:, 8 * rc:8 * rc + 8, :], y[:, 8 * rc:8 * rc + 8, :])
        psum = ctx.enter_context(tc.tile_pool(name="psum", bufs=4, space="PSUM"))
        work = ctx.enter_context(tc.tile_pool(name="work", bufs=4))
        # p-state warmup: dep-free matmuls keep the PE busy through the
        # input DMA window so real matmuls start at full clock
        dum = const.tile([128, 512], BF16)
        nc.vector.memset(dum, 0.0)
        warm = psum.tile([128, 512], F32, tag="ps")
        for i in range(7):
            nc.tensor.matmul(warm, lhsT=ident, rhs=dum,
                             start=(i == 0), stop=(i == 6))
        for r0, nr in RT:
            ps = psum.tile([128, 512], F32, tag="ps")
            n = nr * 96
            nc.tensor.matmul(ps[:, :n], lhsT=s_wc,
                             rhs=s_y[:, 8 + r0:8 + r0 + nr, 8:8 + 96],
                             start=True, stop=False)
            for g in range(4):
                d = DILS[g]
                for i, (ki, kj) in enumerate(TAPS):
                    u0 = 8 + r0 + d * (ki - 1)
                    v0 = 8 + d * (kj - 1)
                    nc.tensor.matmul(
                        ps[32 * g:32 * g + 16, :n],
                        lhsT=s_w[:, g, ki * 3 + kj, :],
                        rhs=s_y[:, u0:u0 + nr, v0:v0 + 96],
                        start=False, stop=(i == 7),
                        tile_position=(0, 32 * g),
                    )
            ob = work.tile([128, 512], F32, tag="ob")
            nc.scalar.activation(ob[:, :n], ps[:, :n],
                                 mybir.ActivationFunctionType.Relu,
                                 bias=s_b)
            nc.sync.dma_start(fo[:, RT.index((r0, nr)), :n], ob[:, :n])
    nc.compile()
    return nc


def _get(name, builder):
    if name not in _cache:
        _cache[name] = builder()
    return _cache[name]


# ---------------------------------------------------------------- entry
def kernel(x1, x2, mask, mask_all, fuse_w, fuse_b, _collect=None):
    x1 = np.asarray(x1, np.float32)
    x2 = np.asarray(x2, np.float32)
    mask = np.asarray(mask, np.float32)
    mask_all = np.asarray(mask_all, np.float32)
    fuse_w = np.asarray(fuse_w, np.float32)
    fuse_b = np.asarray(fuse_b, np.float32)
    N = x1.shape[0]
    bf = ml_dtypes.bfloat16
    f8 = ml_dtypes.float8_e4m3fn

    NB = _neighbor_mask()  # [L, L]

    # ---- per-batch compaction / permutation planning.  p-rows are dealt
    # to cores so each gets an equal share of self-match (non-hole) rows:
    # those go to fp8 stages, hole rows to bf16 stages.
    plans = []
    nlt = 0
    nf8 = 3
    for b in range(N):
        mp = _im2col3(mask[b])
        mmv = (mp.mean(0) == 0.0).astype(np.float32)
        valid = np.where(mmv > 0)[0]
        nlt = max(nlt, (len(valid) + 127) // 128)
        nh_all = np.where(mmv > 0)[0]
        hl_all = np.where(mmv == 0)[0]
        nhs = len(nh_all)
        sizes = [nhs // 4 + (1 if j < nhs % 4 else 0) for j in range(4)]
        perms = []
        no, ho = 0, 0
        for j in range(4):
            nh = nh_all[no:no + sizes[j]]
            no += sizes[j]
            hl = hl_all[ho:ho + CHUNK - sizes[j]]
            ho += CHUNK - sizes[j]
            perms.append((nh, hl))
            nf8 = min(nf8, len(nh) // 128)
        plans.append((mmv, valid, perms))
    NLP = nlt * 128

    in_maps = []
    unperms = []
    corrs = []
    for b in range(N):
        mmv, valid, perms = plans[b]
        P = _im2col3(x2[b])
        norms = np.sqrt((P * P).sum(0))
        ma = mask_all[b, 0].reshape(L)
        lhs_full = P * (SCALE * ma)[None, :]
        rhs_full = P * (mmv / np.maximum(norms, 1e-4))[None, :]
        rhs_c = np.zeros((1152, NLP), np.float32)
        rhs_c[:, :len(valid)] = rhs_full[:, valid]
        rhsb_r = _pack_part(rhs_c, 128).astype(bf)                # [128,9,NLP]
        if nf8:
            rhs8_c = np.zeros((1280, NLP), np.float32)
            rhs8_c[:1152] = rhs_c * F8S
            r8p = _pack_part(rhs8_c, 128).astype(f8)              # [128,10,NLP]
            _, _, SLh = _slices(nlt)
            rhs8_r = np.concatenate(                              # slice-major
                [r8p[:, :, o:o + s].reshape(128, -1) for o, s in SLh], axis=1)
            rhs8_r = np.ascontiguousarray(rhs8_r)                 # [128,10*NLP]
        RW = _im2col4s2(x1[b])                                    # [L, 2048]
        rw_c = np.zeros((NLP, 2048), np.float32)
        rw_c[:len(valid)] = RW[valid]
        rw_r = _pack_part(rw_c, 128).astype(bf)                   # [128,nlt,2048]
        # 1e-8 floor for ALL l (valid via floor-fold, invalid exactly)
        corrs.append(1e-8 * RW.sum(0))
        postF = (1.0 + 0.5 * NB) * mmv[:, None] * ma[None, :]     # [l, p]
        postC = postF[valid]                                      # [nv, p]
        for j in range(4):
            nh, hl = perms[j]
            if nf8:
                perm = np.concatenate([nh, hl])
            else:
                perm = np.concatenate([nh, hl])
            unperms.append(perm)
            lhs_p = lhs_full[:, perm]                             # [1152, 576]
            post_p = np.zeros((CHUNKP, NLP), np.float32)
            post_p[:CHUNK, :len(valid)] = postC[:, perm].T
            im = {
                "rhsb": rhsb_r,
                "post": _pack_part(post_p, 128).astype(bf),
                "rw": rw_r,
            }
            nbf = 5 - nf8
            if nf8:
                im["rhs8"] = rhs8_r
                lf8 = np.zeros((1280, nf8 * 128), np.float32)
                lf8[:1152] = lhs_p[:, :nf8 * 128] / F8S
                l8 = _pack_part(lf8, 128).reshape(128, 10, nf8, 128)
                im["lhs8"] = np.ascontiguousarray(
                    l8.transpose(0, 2, 1, 3)).astype(f8)          # [128,nf8,10,128]
            lbf = np.zeros((1152, nbf * 128), np.float32)
            lbf[:, :CHUNK - nf8 * 128] = lhs_p[:, nf8 * 128:]
            lb = _pack_part(lbf, 128).reshape(128, 9, nbf, 128)
            im["lhsb"] = np.ascontiguousarray(
                lb.transpose(0, 2, 1, 3)).astype(bf)              # [128,nbf,9,128]
            in_maps.append(im)

    nc1 = _get(f"main_{nlt}_{nf8}", lambda: _build_main(nlt, nf8))
    res1 = bass_utils.run_bass_kernel_spmd(nc1, in_maps, core_ids=list(range(8)))
    if _collect is not None:
        _collect.append(res1)

    ys = []
    for b in range(N):
        col = np.empty((L, 2048), np.float32)
        for j in range(4):
            r = res1.results[b * 4 + j]["col"]     # [128, 5, 2048]
            colP = r.transpose(1, 0, 2).reshape(CHUNKP, 2048)[:CHUNK]
            col[unperms[b * 4 + j]] = colP
        col += corrs[b][None, :]
        ys.append(_col2im(col) / 4.0)
    y = np.stack(ys)                               # [N, 128, 96, 96]

    fw_r = np.ascontiguousarray(
        fuse_w.transpose(2, 0, 3, 4, 1).reshape(128, 4, 9, 16)).astype(bf)
    fwc_r = np.zeros((128, 128), np.float32)       # center taps, 32-stride
    fbx_r = np.zeros((128, 1), np.float32)
    for g in range(4):
        fwc_r[:, 32 * g:32 * g + 16] = fuse_w[g, :, :, 1, 1].T
        fbx_r[32 * g:32 * g + 16, 0] = fuse_b[g]
    fwc_r = fwc_r.astype(bf)
    in_maps2 = []
    for b in range(N):
        yp = np.pad(y[b], ((0, 0), (8, 8), (8, 8))).astype(bf)  # [128,112,112]
        for q in range(4):
            in_maps2.append({
                "yslab": np.ascontiguousarray(yp[:, 24 * q:24 * q + 40, :]),
                "fw": fw_r, "fwc": fwc_r, "fbx": fbx_r,
            })
    nc2 = _get("fuse", _build_fuse)
    res2 = bass_utils.run_bass_kernel_spmd(nc2, in_maps2, core_ids=list(range(8)))
    if _collect is not None:
        _collect.append(res2)

    RT = [(0, 5), (5, 5), (10, 5), (15, 5), (20, 4)]
    out = np.empty((N, 64, 96, 96), np.float32)
    for b in range(N):
        for q in range(4):
            r = res2.results[b * 4 + q]["fo"]      # [128, 5, 512]
            for g in range(4):
                for rt, (r0, nr) in enumerate(RT):
                    blk = r[32 * g:32 * g + 16, rt, :nr * 96].reshape(16, nr, 96)
                    out[b, 16 * g:16 * g + 16, 24 * q + r0:24 * q + r0 + nr, :] = blk
    return out
